# revision 5
# baseline (speedup 1.0000x reference)
"""Trainium2 Bass kernel for nn_Attention_39676907884025.

Reference semantics: q_param (a scalar) is broadcast over both query and key,
so the score matrix qk[b,q,k] = sum_d p*p is CONSTANT along the softmax axis.
Softmax of a constant row is exactly uniform (x - max(x) == 0 bit-exactly,
exp(0) == 1, sum == SK exactly, 1/SK is a power of two), so

    out[b, q, :] = (1/SK) * sum_k value[b, k, :]     for every q.

query / key / q_param never need to touch the device.

Distribution: data-parallel over batch B=16 across 8 NeuronCores (2 batches
per core). Per core and batch:
  1. one 1MB DMA load of value[b] (2048, 128) into SBUF laid out as
     (p=128 partitions, 16*128 free) with k = t*128 + p,
  2. an exact fp32 DVE add-tree folding the 16 k-tiles -> (128, 128),
  3. one fp32 matmul with a constant (1/2048) matrix as stationary weights:
     psum[q, d] = sum_p (1/2048) * acc[p, d] -- reduces across partitions AND
     broadcasts the mean row to all 128 q-partitions in one shot,
  4. 16 x 64KB DMA stores of that tile into the output rows.
"""

import sys

import numpy as np

if "/opt/trn_rl_repo" not in sys.path:
    sys.path.insert(0, "/opt/trn_rl_repo")

B, SQ, SK, D, DV = 16, 2048, 2048, 128, 128
N_CORES = 8
BPC = B // N_CORES  # batches per core
P = 128

LAST_RESULT = None  # BassKernelResults of the most recent run (for profiling)


def _build_nc():
    import concourse.bacc as bacc
    import concourse.mybir as mybir
    from concourse.tile import TileContext

    f32 = mybir.dt.float32
    nc = bacc.Bacc("TRN2", target_bir_lowering=False)

    val = nc.dram_tensor("value", [BPC, SK, DV], f32, kind="ExternalInput")
    out = nc.dram_tensor("out", [BPC, SQ, DV], f32, kind="ExternalOutput")

    nt = SK // P  # 16 k-tiles per batch
    nq = SQ // P  # 16 q-tiles per batch

    with TileContext(nc) as tc:
        with (
            tc.tile_pool(name="x", bufs=2) as xpool,
            tc.tile_pool(name="tree", bufs=2) as tpool,
            tc.tile_pool(name="const", bufs=1) as cpool,
            tc.tile_pool(name="psum", bufs=2, space="PSUM") as ppool,
        ):
            # Stationary matmul operand: every entry 1/SK (exact power of 2),
            # so the partition-reduction matmul also applies the softmax
            # weight exactly.
            w = cpool.tile([P, P], f32)
            nc.vector.memset(w[:], 1.0 / SK)

            for b in range(BPC):
                # SBUF xt[p, t*128 + d] = value[b, t*128 + p, d]
                xt = xpool.tile([P, SK], f32)
                nc.sync.dma_start(
                    xt[:].rearrange("p (t d) -> p t d", d=DV),
                    val[b].rearrange("(t p) d -> p t d", p=P),
                )

                # Exact fp32 free-axis add tree over the 16 k-tiles.
                t1 = tpool.tile([P, 1024], f32, tag="t1")
                nc.vector.tensor_add(t1[:], xt[:, 0:1024], xt[:, 1024:2048])
                t2 = tpool.tile([P, 512], f32, tag="t2")
                nc.vector.tensor_add(t2[:], t1[:, 0:512], t1[:, 512:1024])
                t3 = tpool.tile([P, 256], f32, tag="t3")
                nc.vector.tensor_add(t3[:], t2[:, 0:256], t2[:, 256:512])
                t4 = tpool.tile([P, P], f32, tag="t4")
                nc.vector.tensor_add(t4[:], t3[:, 0:128], t3[:, 128:256])

                # psum[q, d] = sum_p (1/SK) * t4[p, d]  for all q rows.
                ps = ppool.tile([P, P], f32)
                nc.tensor.matmul(ps[:], w[:], t4[:], start=True, stop=True)

                # Replicate the (128, 128) mean tile 16x along the free axis
                # by doubling copies, so the whole batch output is one DMA.
                wide = xpool.tile([P, SQ], f32, tag="wide")
                nc.vector.tensor_copy(wide[:, 0:P], ps[:])
                width = P
                while width < SQ:
                    nc.vector.tensor_copy(
                        wide[:, width : 2 * width], wide[:, 0:width]
                    )
                    width *= 2

                nc.sync.dma_start(
                    out[b].rearrange("(t p) d -> p t d", p=P),
                    wide[:].rearrange("p (t d) -> p t d", d=DV),
                )

    nc.compile()
    return nc


def kernel(query=None, key=None, value=None, q_param=None, _trace=False):
    from concourse.bass_utils import run_bass_kernel_spmd

    global LAST_RESULT

    value = np.ascontiguousarray(np.asarray(value, dtype=np.float32))
    assert value.shape == (B, SK, DV), value.shape

    nc = _build_nc()
    shards = value.reshape(N_CORES, BPC, SK, DV)
    in_maps = [{"value": shards[i]} for i in range(N_CORES)]

    LAST_RESULT = run_bass_kernel_spmd(
        nc, in_maps, list(range(N_CORES)), trace=_trace
    )
    return np.concatenate(
        [LAST_RESULT.results[i]["out"] for i in range(N_CORES)], axis=0
    )


# revision 7
# speedup vs baseline: 1.1408x; 1.1408x over previous
"""Trainium2 Bass kernel for nn_Attention_39676907884025.

Reference semantics: q_param (a scalar) is broadcast over both query and key,
so the score matrix qk[b,q,k] = sum_d p*p is CONSTANT along the softmax axis.
Softmax of a constant row is exactly uniform (x - max(x) == 0 bit-exactly,
exp(0) == 1, sum == SK exactly, 1/SK is a power of two), so

    out[b, q, :] = (1/SK) * sum_k value[b, k, :]     for every q.

query / key / q_param never need to touch the device.

Distribution: data-parallel over batch B=16 across 8 NeuronCores (2 batches
per core). Per core and batch:
  1. one 1MB DMA load of value[b] (2048, 128) into SBUF laid out as
     (p=128 partitions, 16*128 free) with k = t*128 + p,
  2. an exact fp32 DVE add-tree folding the 16 k-tiles -> (128, 128),
  3. one fp32 matmul with a constant (1/2048) matrix as stationary weights:
     psum[q, d] = sum_p (1/2048) * acc[p, d] -- reduces across partitions AND
     broadcasts the mean row to all 128 q-partitions in one shot,
  4. 16 x 64KB DMA stores of that tile into the output rows.
"""

import sys

import numpy as np

if "/opt/trn_rl_repo" not in sys.path:
    sys.path.insert(0, "/opt/trn_rl_repo")

B, SQ, SK, D, DV = 16, 2048, 2048, 128, 128
N_CORES = 8
BPC = B // N_CORES  # batches per core
P = 128

LAST_RESULT = None  # BassKernelResults of the most recent run (for profiling)


def _build_nc():
    import concourse.bacc as bacc
    import concourse.mybir as mybir
    from concourse.tile import TileContext

    f32 = mybir.dt.float32
    nc = bacc.Bacc("TRN2", target_bir_lowering=False)

    val = nc.dram_tensor("value", [BPC, SK, DV], f32, kind="ExternalInput")
    out = nc.dram_tensor("out", [BPC, SQ, DV], f32, kind="ExternalOutput")

    nt = SK // P  # 16 k-tiles per batch
    nq = SQ // P  # 16 q-tiles per batch

    with TileContext(nc) as tc:
        with (
            tc.tile_pool(name="x", bufs=2) as xpool,
            tc.tile_pool(name="tree", bufs=2) as tpool,
            tc.tile_pool(name="const", bufs=1) as cpool,
            tc.tile_pool(name="psum", bufs=2, space="PSUM") as ppool,
        ):
            # Stationary matmul operand: every entry 1/SK (exact power of 2),
            # so the partition-reduction matmul also applies the softmax
            # weight exactly.
            w = cpool.tile([P, P], f32)
            nc.vector.memset(w[:], 1.0 / SK)

            for b in range(BPC):
                # SBUF xt[p, t*128 + d] = value[b, p*16 + t, d]: each
                # partition owns 16 consecutive DRAM rows (8KB contiguous),
                # and the load is split across both HWDGE queues (Sync/Act).
                xt = xpool.tile([P, SK], f32)
                xdst = xt[:].rearrange("p (t d) -> p t d", d=DV)
                xsrc = val[b].rearrange("(p t) d -> p t d", p=P)
                half = SK // P // 2  # 8 row-tiles per queue
                nc.sync.dma_start(xdst[:, 0:half, :], xsrc[:, 0:half, :])
                nc.scalar.dma_start(
                    xdst[:, half : 2 * half, :], xsrc[:, half : 2 * half, :]
                )

                # Exact fp32 free-axis add tree over the 16 k-tiles.
                t1 = tpool.tile([P, 1024], f32, tag="t1")
                nc.vector.tensor_add(t1[:], xt[:, 0:1024], xt[:, 1024:2048])
                t2 = tpool.tile([P, 512], f32, tag="t2")
                nc.vector.tensor_add(t2[:], t1[:, 0:512], t1[:, 512:1024])
                t3 = tpool.tile([P, 256], f32, tag="t3")
                nc.vector.tensor_add(t3[:], t2[:, 0:256], t2[:, 256:512])
                t4 = tpool.tile([P, P], f32, tag="t4")
                nc.vector.tensor_add(t4[:], t3[:, 0:128], t3[:, 128:256])

                # psum[q, d] = sum_p (1/SK) * t4[p, d]  for all q rows.
                ps = ppool.tile([P, P], f32)
                nc.tensor.matmul(ps[:], w[:], t4[:], start=True, stop=True)

                # Replicate the (128, 128) mean tile 8x along the free axis
                # by doubling copies; each of the two stores (one per HWDGE
                # queue) reads it once, covering 8 row-tiles of the output.
                wide = xpool.tile([P, SQ // 2], f32, tag="wide")
                nc.vector.tensor_copy(wide[:, 0:P], ps[:])
                width = P
                while width < SQ // 2:
                    nc.vector.tensor_copy(
                        wide[:, width : 2 * width], wide[:, 0:width]
                    )
                    width *= 2

                odst = out[b].rearrange("(p t) d -> p t d", p=P)
                wsrc = wide[:].rearrange("p (t d) -> p t d", d=DV)
                nc.sync.dma_start(odst[:, 0:half, :], wsrc)
                nc.scalar.dma_start(odst[:, half : 2 * half, :], wsrc)

    nc.compile()
    return nc


def kernel(query=None, key=None, value=None, q_param=None, _trace=False):
    from concourse.bass_utils import run_bass_kernel_spmd

    global LAST_RESULT

    value = np.ascontiguousarray(np.asarray(value, dtype=np.float32))
    assert value.shape == (B, SK, DV), value.shape

    nc = _build_nc()
    shards = value.reshape(N_CORES, BPC, SK, DV)
    in_maps = [{"value": shards[i]} for i in range(N_CORES)]

    LAST_RESULT = run_bass_kernel_spmd(
        nc, in_maps, list(range(N_CORES)), trace=_trace
    )
    return np.concatenate(
        [LAST_RESULT.results[i]["out"] for i in range(N_CORES)], axis=0
    )


# revision 8
# speedup vs baseline: 1.1969x; 1.0492x over previous
"""Trainium2 Bass kernel for nn_Attention_39676907884025.

Reference semantics: q_param (a scalar) is broadcast over both query and key,
so the score matrix qk[b,q,k] = sum_d p*p is CONSTANT along the softmax axis.
Softmax of a constant row is exactly uniform (x - max(x) == 0 bit-exactly,
exp(0) == 1, sum == SK exactly, 1/SK is a power of two), so

    out[b, q, :] = (1/SK) * sum_k value[b, k, :]     for every q.

query / key / q_param never need to touch the device.

Distribution: data-parallel over batch B=16 across 8 NeuronCores (2 batches
per core). Per core and batch:
  1. one 1MB DMA load of value[b] (2048, 128) into SBUF laid out as
     (p=128 partitions, 16*128 free) with k = t*128 + p,
  2. an exact fp32 DVE add-tree folding the 16 k-tiles -> (128, 128),
  3. one fp32 matmul with a constant (1/2048) matrix as stationary weights:
     psum[q, d] = sum_p (1/2048) * acc[p, d] -- reduces across partitions AND
     broadcasts the mean row to all 128 q-partitions in one shot,
  4. 16 x 64KB DMA stores of that tile into the output rows.
"""

import sys

import numpy as np

if "/opt/trn_rl_repo" not in sys.path:
    sys.path.insert(0, "/opt/trn_rl_repo")

B, SQ, SK, D, DV = 16, 2048, 2048, 128, 128
N_CORES = 8
BPC = B // N_CORES  # batches per core
P = 128

LAST_RESULT = None  # BassKernelResults of the most recent run (for profiling)


def _build_nc():
    import concourse.bacc as bacc
    import concourse.mybir as mybir
    from concourse.tile import TileContext

    f32 = mybir.dt.float32
    nc = bacc.Bacc("TRN2", target_bir_lowering=False)

    val = nc.dram_tensor("value", [BPC, SK, DV], f32, kind="ExternalInput")
    out = nc.dram_tensor("out", [BPC, SQ, DV], f32, kind="ExternalOutput")

    nt = SK // P  # 16 k-tiles per batch
    nq = SQ // P  # 16 q-tiles per batch

    with TileContext(nc) as tc:
        with (
            tc.tile_pool(name="x", bufs=2) as xpool,
            tc.tile_pool(name="tree", bufs=2) as tpool,
            tc.tile_pool(name="const", bufs=1) as cpool,
            tc.tile_pool(name="psum", bufs=2, space="PSUM") as ppool,
        ):
            # Stationary matmul operand: every entry 1/SK (exact power of 2),
            # so the partition-reduction matmul also applies the softmax
            # weight exactly.
            w = cpool.tile([P, P], f32)
            nc.vector.memset(w[:], 1.0 / SK)

            # Queue pick per chunk index: even -> Sync HWDGE, odd -> Act HWDGE.
            dma_eng = [nc.sync, nc.scalar]

            for b in range(BPC):
                # SBUF xt[p, t*128 + d] = value[b, p*16 + t, d]: each
                # partition owns 16 consecutive DRAM rows (8KB contiguous).
                # Load in 4 quarter-chunks (256KB each, alternating HWDGE
                # queues) and reduce each quarter as soon as it lands, so
                # DVE work overlaps the remaining loads.
                xt = xpool.tile([P, SK], f32)
                xdst = xt[:].rearrange("p (t d) -> p t d", d=DV)
                xsrc = val[b].rearrange("(p t) d -> p t d", p=P)

                accs = []
                for qi in range(4):
                    t0, t1 = 4 * qi, 4 * (qi + 1)
                    dma_eng[qi % 2].dma_start(
                        xdst[:, t0:t1, :], xsrc[:, t0:t1, :]
                    )
                    lo, hi = 512 * qi, 512 * (qi + 1)
                    a = tpool.tile([P, 256], f32, tag=f"a{qi % 2}")
                    nc.vector.tensor_add(
                        a[:], xt[:, lo : lo + 256], xt[:, lo + 256 : hi]
                    )
                    acc = tpool.tile([P, P], f32, tag=f"acc{qi}")
                    nc.vector.tensor_add(acc[:], a[:, 0:128], a[:, 128:256])
                    accs.append(acc)

                s01 = tpool.tile([P, P], f32, tag="s01")
                nc.vector.tensor_add(s01[:], accs[0][:], accs[1][:])
                s23 = tpool.tile([P, P], f32, tag="s23")
                nc.vector.tensor_add(s23[:], accs[2][:], accs[3][:])
                t4 = tpool.tile([P, P], f32, tag="t4")
                nc.vector.tensor_add(t4[:], s01[:], s23[:])

                # psum[q, d] = sum_p (1/SK) * t4[p, d]  for all q rows.
                ps = ppool.tile([P, P], f32)
                nc.tensor.matmul(ps[:], w[:], t4[:], start=True, stop=True)

                # Replicate the mean tile 4x along the free axis; each of the
                # 4 stores (alternating queues) reads it, covering 4 output
                # row-tiles (256KB) apiece.
                wide = xpool.tile([P, 512], f32, tag="wide")
                nc.vector.tensor_copy(wide[:, 0:P], ps[:])
                nc.vector.tensor_copy(wide[:, P : 2 * P], wide[:, 0:P])
                nc.vector.tensor_copy(wide[:, 2 * P : 4 * P], wide[:, 0 : 2 * P])

                odst = out[b].rearrange("(p t) d -> p t d", p=P)
                wsrc = wide[:].rearrange("p (t d) -> p t d", d=DV)
                for qi in range(4):
                    t0, t1 = 4 * qi, 4 * (qi + 1)
                    dma_eng[qi % 2].dma_start(odst[:, t0:t1, :], wsrc)

    nc.compile()
    return nc


def kernel(query=None, key=None, value=None, q_param=None, _trace=False):
    from concourse.bass_utils import run_bass_kernel_spmd

    global LAST_RESULT

    value = np.ascontiguousarray(np.asarray(value, dtype=np.float32))
    assert value.shape == (B, SK, DV), value.shape

    nc = _build_nc()
    shards = value.reshape(N_CORES, BPC, SK, DV)
    in_maps = [{"value": shards[i]} for i in range(N_CORES)]

    LAST_RESULT = run_bass_kernel_spmd(
        nc, in_maps, list(range(N_CORES)), trace=_trace
    )
    return np.concatenate(
        [LAST_RESULT.results[i]["out"] for i in range(N_CORES)], axis=0
    )
